# revision 16
# baseline (speedup 1.0000x reference)
"""Trainium2 Bass kernel for the disentangled non-local block.

Per batch b (one NeuronCore each, 8 batches over 8 cores):
  x: [64, 4096]; q/k/v = 1x1 conv GEMMs; out = x + softmax(q'k/8)V + unary.

Design (measured ~166us/iter on HW vs ~267us for the fp32r baseline):
  - All-bf16 GEMM datapath.  Biases are algebraically removed: bq/bk
    cancel in the softmax over keys (only qbar keeps bq); bv contributes
    exactly +2*bv[c] per output pixel (b_v*(1^T E) = b_v*D cancels /D).
  - x, q, k duplicated onto partitions 64..127 so S matmuls run as
    row-packed concurrent pairs (two 64-contraction tiles at once via
    tile_position auto-derive) -- 2x PE throughput on the S stream.
    The duplication is free: GEMM weights are stored twice ([wq|wq]).
  - exp of the [128 m, 1024 n] logit tiles split across ScalarE (native
    Exp with per-partition bias -u_m/8) and DVE via a one-instruction
    Schraudolph fast-exp: int8 out = s*1.44269 + (56.0368 - 1.44269*u_m)
    whose bits are the fp8e4m3 approximation of exp((s-u_m)/8);
    zero-mean error that cancels through the softmax normalization.
    (GPSIMD cannot read PSUM on this HW, so Pool takes no exp tiles.)
  - O accumulation via fp8e4m3 DoubleRow matmuls: vT stored interleaved
    [128, pair, 2, 80] with chunk pairs 2-per-cell, so each O matmul
    contracts 256 keys in 512 cycles -- 2x the bf16 O throughput.  The
    ones column of vT (at index 64) accumulates the softmax denominator
    at PSUM partition 64 for free.  The unary path reuses the fp8 vT
    with eu scaled by 1/16 (a common scale cancels in out_u/du).
  - Single PSUM O accumulator per n-block (psS 3x[128,1024] + psO 1):
    O-matmul emission lags S/exp and waits for the staged epilogue of
    the previous block, so the PE queue never head-of-line blocks.
  - Epilogue staged across the next block's rounds (recip+bcast mm,
    bcs copy, mul+add+DMA) to keep the DVE queue from stalling; the
    last block runs a 512-wide minimum-latency tail instead.
  - Big SBUF tiles double-buffered across the repeat loop so back-to-
    back iterations overlap (repeat-delta HW timing measures this).
"""

import numpy as np

B = 8
CIN = 64
C = 64
H = W = 64
N = H * W            # 4096
NB = 1024            # n-block (columns per outer iteration)
NBLK = N // NB       # 4
MB = 128             # m-chunk (keys per matmul, partition dim)
MCH = N // MB        # 32
HALF = 512           # PSUM bank free-dim limit for fp32 matmul output
SCALE = 0.125        # 1 / (sqrt(C) * temperature)
C0S = 23.0831        # SCALE * 2^7 / ln2   (Schraudolph slope, bf16)
C1S = 16249.08       # 127*2^7 + 0.5 - 0.0579*2^7 (trunc + centering)
C0S8 = 1.44269       # SCALE * 2^3 / ln2   (Schraudolph slope, fp8e4m3)
C1S8 = 56.0368       # 7*2^3 + 0.5 - 0.0579*2^3

# exp-tile engine assignment weights (ACT, DVE, POOL) out of the 128
# [128,1024] logit tiles.  Tuned so ACT and DVE busy-times balance.
import os as _os
W_ACT = int(_os.environ.get("W_ACT", "74"))
W_DVE = int(_os.environ.get("W_DVE", str(128 - int(_os.environ.get("W_ACT", "74")))))
W_POOL = 0   # Pool can't read PSUM on HW

_CACHE = {}
ABLATE = ""


def _exp_pattern():
    """Deterministic interleave of 'A'/'D'/'P' with the global ratios."""
    pat = []
    cnt = {"A": 0, "D": 0, "P": 0}
    wts = {"A": W_ACT, "D": W_DVE, "P": W_POOL}
    tot = W_ACT + W_DVE + W_POOL
    for i in range(128):
        best = max("ADP", key=lambda e: wts[e] * (i + 1) / tot - cnt[e])
        cnt[best] += 1
        pat.append(best)
    return pat


def _build(repeat=1, compat=True, ablate=None):
    import concourse.bass as bass
    import concourse.tile as tile
    from concourse import mybir

    f32 = mybir.dt.float32
    bf16 = mybir.dt.bfloat16
    i16 = mybir.dt.int16
    fp8 = mybir.dt.float8e4
    i8 = mybir.dt.int8
    AX = mybir.AxisListType
    AF = mybir.ActivationFunctionType
    ALU = mybir.AluOpType

    abl = ABLATE if ablate is None else ablate
    out_eng_name = _os.environ.get("OUT_Q", "gpsimd")
    nc = bass.Bass()
    x2_d = nc.dram_tensor("x2", [2 * CIN, N], bf16, kind="ExternalInput")
    xf_d = nc.dram_tensor("xf", [C, N], f32, kind="ExternalInput")
    wqk_d = nc.dram_tensor("wqk", [2 * CIN, 2 * C], bf16, kind="ExternalInput")
    wv2_d = nc.dram_tensor("wv2", [2 * CIN, C], bf16, kind="ExternalInput")
    bq2_d = nc.dram_tensor("bq2", [2 * C, 1], f32, kind="ExternalInput")
    bvx_d = nc.dram_tensor("bvx", [C, 1], f32, kind="ExternalInput")
    out_d = nc.dram_tensor("out", [C, N], f32, kind="ExternalOutput")

    pat = _exp_pattern()

    with tile.TileContext(nc) as tc:
        with (
            tc.tile_pool(name="sing", bufs=1) as sing,
            tc.tile_pool(name="dbl", bufs=2) as dbl,
            tc.tile_pool(name="epool", bufs=12) as epool,
            tc.tile_pool(name="dpool", bufs=2) as dpool,
            tc.tile_pool(name="opool", bufs=2) as opool,
            tc.tile_pool(name="psS", bufs=3, space="PSUM") as psS,
            tc.tile_pool(name="psO", bufs=1, space="PSUM") as psO,
        ):
          for _rep in range(repeat):
            # ---- input DMAs (split across queues) ----
            wqk_sb = sing.tile([2 * CIN, 2 * C], bf16)
            wv2_sb = sing.tile([2 * CIN, C], bf16)
            bq2_sb = sing.tile([2 * C, 1], f32)
            bvx_sb = sing.tile([C, 1], f32)
            nc.gpsimd.dma_start(out=wqk_sb, in_=wqk_d[:])
            nc.gpsimd.dma_start(out=wv2_sb, in_=wv2_d[:])
            nc.gpsimd.dma_start(out=bq2_sb, in_=bq2_d[:])
            nc.gpsimd.dma_start(out=bvx_sb, in_=bvx_d[:])
            x2_sb = dbl.tile([2 * CIN, N], bf16, tag="x2_sb")
            for qd in range(4):
                eng = nc.sync if qd % 2 == 0 else nc.gpsimd
                qs = slice(qd * (N // 4), (qd + 1) * (N // 4))
                eng.dma_start(out=x2_sb[:, qs], in_=x2_d[:, qs])
            xf_sb = dbl.tile([C, N], f32, tag="xf_sb")
            for h in range(2):
                eng = nc.sync if h == 0 else nc.gpsimd
                eng.dma_start(
                    out=xf_sb[:, h * (N // 2):(h + 1) * (N // 2)],
                    in_=xf_d[:, h * (N // 2):(h + 1) * (N // 2)])
            # ---- input-independent init, emitted first so DVE/ACT work
            # while the input DMAs are in flight.  The dummy activation
            # pre-loads the ACT function table (Exp/Copy/Identity share
            # one set) so the first real copy doesn't pay the ~1.3us
            # table load.
            ones_sb = sing.tile([1, C], bf16)
            nc.vector.memset(ones_sb, 1.0)
            vT_sb = dbl.tile([MB, MCH // 2, 2, 80], fp8, tag="vT_sb")
            if _os.environ.get("NARROW_MEMSET", "1") == "1":
                nc.vector.memset(vT_sb[:, :, :, C:C + 1], 1.0)
            else:
                nc.vector.memset(vT_sb, 1.0)
            eub = sing.tile([MB, 1], f32)
            nc.vector.memset(eub, -2.772589)
            qmean2 = sing.tile([MB, 2], bf16)
            nc.vector.memset(qmean2, 0.0)
            if _os.environ.get("WARM", "1") == "1":
                warm_sb = sing.tile([1, 1], f32)
                nc.scalar.activation(out=warm_sb, in_=ones_sb[0:1, 0:1],
                                     func=AF.Exp)

            # ---- q,k GEMMs: row-packed concurrent pairs ----
            # rows 0-63 of wqk = [wqT|wqT]  -> q duplicated on 128 parts
            # rows 64-127     = [wkT|wkT]   -> k duplicated
            # q first (all of qbar's deps land early -> ubias -> first exp
            # starts sooner); copies merged to 1024-wide (2-bank PSUM
            # tiles) to halve per-op overheads.
            q2_sb = dbl.tile([MB, N], bf16, tag="q2_sb")
            k2_sb = dbl.tile([MB, N], bf16, tag="k2_sb")
            qsum8 = dbl.tile([MB, 4], f32, tag="qsum8")
            for h in range(N // NB):
                hs = slice(h * NB, (h + 1) * NB)
                qp = psS.tile([MB, NB], f32, tag="S")
                for g in range(2):
                    gs = slice(h * NB + g * HALF, h * NB + (g + 1) * HALF)
                    nc.tensor.matmul(qp[:, g * HALF:(g + 1) * HALF],
                                     wqk_sb[0:CIN, :], x2_sb[0:CIN, gs],
                                     start=True, stop=True)
                # PSUM->SBUF bf16 copies, alternated across ACT/DVE so
                # neither engine serializes the copy phase; both paths
                # accumulate the row-sum for qbar.
                if h % 2 == 0:
                    nc.scalar.activation(out=q2_sb[:, hs], in_=qp,
                                         func=AF.Copy,
                                         accum_out=qsum8[:, h:h + 1])
                else:
                    nc.vector.tensor_scalar(
                        out=q2_sb[:, hs], in0=qp, scalar1=1.0, scalar2=0.0,
                        op0=ALU.mult, op1=ALU.add,
                        accum_out=qsum8[:, h:h + 1])
            # qbar right after the q phase so the u-matmul inputs are
            # ready by the time the k phase drains.
            qsum = sing.tile([MB, 1], f32)
            nc.vector.reduce_sum(qsum, qsum8, axis=AX.X)
            nc.vector.tensor_scalar(
                out=qmean2[:, 0:1], in0=qsum, scalar1=1.0 / N,
                scalar2=bq2_sb, op0=ALU.mult, op1=ALU.add)

            # ---- k GEMMs with the v GEMMs interleaved ----
            # v chunks ride the gaps of the k copy cadence instead of
            # forming a serial ~6us PE/DVE phase before the S loop.
            for h in range(N // NB):
                hs = slice(h * NB, (h + 1) * NB)
                kp = psS.tile([MB, NB], f32, tag="S")
                for g in range(2):
                    gs = slice(h * NB + g * HALF, h * NB + (g + 1) * HALF)
                    nc.tensor.matmul(kp[:, g * HALF:(g + 1) * HALF],
                                     wqk_sb[CIN:2 * CIN, :],
                                     x2_sb[CIN:2 * CIN, gs],
                                     start=True, stop=True)
                if h % 2 == 0:
                    nc.vector.tensor_copy(k2_sb[:, hs], kp)
                else:
                    nc.scalar.activation(out=k2_sb[:, hs], in_=kp,
                                         func=AF.Copy)
                pass  # v GEMMs are emitted inside block 0's rounds

            # ---- unary logits u = qbar . k ----
            u_ps = psS.tile([MB, 2 * MCH], f32, tag="S")
            for t in range(MCH):
                ms = slice(t * MB, (t + 1) * MB)
                rows = slice(0, CIN) if t % 2 == 0 else slice(CIN, 2 * CIN)
                nc.tensor.matmul(
                    u_ps[:, 2 * t:2 * t + 2], k2_sb[rows, ms],
                    qmean2[rows, :], start=True, stop=True)
            u_even = u_ps[:].rearrange("p (t two) -> p t two", two=2)[:, :, 0:1]
            # exp biases: ACT wants -SCALE*u ; DVE/Pool want C1S - C0S*u
            ubias_a = dbl.tile([MB, MCH], f32, tag="ubias_a")
            nc.scalar.mul(ubias_a, u_even, -SCALE)
            ubias_d = dbl.tile([MB, MCH], f32, tag="ubias_d")
            nc.vector.tensor_scalar(
                out=ubias_d, in0=u_even, scalar1=-C0S8, scalar2=C1S8,
                op0=ALU.mult, op1=ALU.add)
            eu = dbl.tile([MB, 2 * MCH], fp8, tag="eu")
            nc.scalar.activation(out=eu, in_=u_ps, func=AF.Exp,
                                 bias=eub)  # exp(u)/16: du scale cancels


            # ---- unary epilogue (emitted right after block 0) ----
            def emit_unary():
                uacc = psS.tile([C + 1, 2], f32, tag="S")
                for t in range(MCH):
                    nc.tensor.matmul(
                        uacc, vT_sb[:, t // 2, t % 2, 0:C + 1],
                        eu[:, 2 * t:2 * t + 2],
                        start=(t == 0), stop=(t == MCH - 1))
                du = sing.tile([1, 1], f32)
                nc.vector.tensor_copy(du, uacc[C:C + 1, 0:1])
                recu = sing.tile([1, 1], f32)
                nc.vector.reciprocal(recu, du)
                bcu = sing.tile([C, 1], f32)
                rau = recu[:]
                nc.sync.dma_start(out=bcu, in_=bass.AP(
                    tensor=rau.tensor, offset=rau.offset,
                    ap=[rau.ap[0], [0, C], [1, 1]]))
                ucp = sing.tile([C, 1], f32)
                nc.vector.tensor_copy(ucp, uacc[0:C, 0:1])
                ub_sb = sing.tile([C, 1], f32)
                nc.vector.tensor_mul(ub_sb, ucp, bcu)
                ubv = sing.tile([C, 1], f32)
                nc.vector.tensor_add(ubv, ub_sb, bvx_sb)
                # xpu = x + (unary + 2bv), broadcast along n (ACT:
                # real-HW gpsimd elementwise is far slower than modeled)
                xpu = sing.tile([C, N], f32)
                nc.scalar.activation(out=xpu, in_=xf_sb, func=AF.Identity,
                                     bias=ubv, scale=1.0)
                return xpu

            # ---- main loop ----
            out_sb = dbl.tile([C, N], f32, tag="out")
            xpu = None
            expi = [0]       # global exp-tile counter (for pattern)

            def emit_exp(s_ps, t, e8):
                pl = t % 2
                eng = pat[expi[0] % 128]
                expi[0] += 1
                if eng == "A":
                    nc.scalar.activation(out=e8[:, pl, :], in_=s_ps,
                                         func=AF.Exp, scale=SCALE,
                                         bias=ubias_a[:, t:t + 1])
                else:
                    nc.vector.tensor_scalar(
                        out=e8.bitcast(i8)[:, pl, :], in0=s_ps, scalar1=C0S8,
                        scalar2=ubias_d[:, t:t + 1],
                        op0=ALU.mult, op1=ALU.add)

            def make_epilogue(j, o_ps):
                """Stage list; each stage is one engine-chain step so the
                DVE queue never head-of-line-blocks on a long chain."""
                def st_recip():
                    rec = dpool.tile([1, NB], bf16, tag="rec")
                    with nc.allow_low_precision(reason="1/D bcast bf16"):
                        nc.vector.reciprocal(rec, o_ps[C:C + 1, :])
                    for h in range(NB // HALF):
                        hs = slice(h * HALF, (h + 1) * HALF)
                        nc.tensor.matmul(
                            o_ps[C:2 * C, hs], ones_sb, rec[:, hs],
                            start=True, stop=True, tile_position=(0, C))

                def st_bcs():
                    bcs = opool.tile([C, NB], f32, tag="bcs")
                    nc.vector.tensor_copy(bcs, o_ps[C:2 * C, :])
                    st_bcs.bcs = bcs

                def st_fin():
                    gs = slice(j * NB, (j + 1) * NB)
                    o_sb = out_sb[:, gs]
                    nc.vector.tensor_mul(o_sb, o_ps[0:C, :], st_bcs.bcs)
                    nc.vector.tensor_add(o_sb, o_sb, xpu[:, gs])
                    if abl != "noout":
                        getattr(nc, out_eng_name).dma_start(out=out_d[:, gs], in_=o_sb)

                def tail():
                    # last block: two 512-half chains interleaved; the bcs
                    # copies go to ACT (idle at this point) so the DVE
                    # serial chain is recip/mul/add only.
                    hs_l = [slice(h * HALF, (h + 1) * HALF)
                            for h in range(NB // HALF)]
                    gs_l = [slice(j * NB + h * HALF, j * NB + (h + 1) * HALF)
                            for h in range(NB // HALF)]
                    recs, bcss = [], []
                    for h in range(NB // HALF):
                        rec = dpool.tile([1, HALF], bf16, tag="rec")
                        with nc.allow_low_precision(reason="1/D bcast bf16"):
                            nc.vector.reciprocal(rec, o_ps[C:C + 1, hs_l[h]])
                        recs.append(rec)
                        nc.tensor.matmul(
                            o_ps[C:2 * C, hs_l[h]], ones_sb, rec,
                            start=True, stop=True, tile_position=(0, C))
                    for h in range(NB // HALF):
                        bcs = opool.tile([C, HALF], f32, tag="bcs")
                        nc.scalar.activation(out=bcs, in_=o_ps[C:2 * C, hs_l[h]],
                                             func=AF.Copy)
                        bcss.append(bcs)
                    for h in range(NB // HALF):
                        o_sb = out_sb[:, gs_l[h]]
                        nc.vector.tensor_mul(o_sb, o_ps[0:C, hs_l[h]], bcss[h])
                        nc.vector.tensor_add(o_sb, o_sb, xpu[:, gs_l[h]])
                        if abl != "noout":
                            getattr(nc, out_eng_name).dma_start(out=out_d[:, gs_l[h]], in_=o_sb)

                if j == NBLK - 1:
                    return [tail]
                return [st_recip, st_bcs, st_fin]

            pending_stages = []
            for j in range(NBLK):
                o_ps = None
                pend_O = []      # (t, e_sb) waiting for O emission
                first_t = [True]

                def drain_O(n=2):
                    nonlocal o_ps
                    for _ in range(min(n, len(pend_O))):
                        rp, e8 = pend_O.pop(0)
                        if o_ps is None:
                            o_ps = psO.tile([MB, NB], f32, tag="O")
                        for h in range(NB // HALF):
                            hs = slice(h * HALF, (h + 1) * HALF)
                            nc.tensor.matmul(
                                o_ps[0:C + 1, hs],
                                vT_sb[:, rp, :, 0:C + 1], e8[:, :, hs],
                                start=first_t[0], stop=(rp == MCH // 2 - 1),
                                perf_mode=mybir.MatmulPerfMode.DoubleRow)
                        first_t[0] = False

                for r in range(MCH // 2):
                    tA, tB = 2 * r, 2 * r + 1
                    s_psA = psS.tile([MB, NB], f32, tag="S")
                    s_psB = psS.tile([MB, NB], f32, tag="S")
                    for h in range(NB // HALF):
                        hs = slice(h * HALF, (h + 1) * HALF)
                        nc.tensor.matmul(
                            s_psA[:, hs], k2_sb[0:CIN, tA * MB:(tA + 1) * MB],
                            q2_sb[0:CIN, j * NB + h * HALF:
                                  j * NB + (h + 1) * HALF],
                            start=True, stop=True)
                        nc.tensor.matmul(
                            s_psB[:, hs],
                            k2_sb[CIN:2 * CIN, tB * MB:(tB + 1) * MB],
                            q2_sb[CIN:2 * CIN, j * NB + h * HALF:
                                  j * NB + (h + 1) * HALF],
                            start=True, stop=True)
                    if j == 0:
                        # v chunk pair r rides round r: its PSUM tile takes
                        # the same ring slot every round, its matmuls are
                        # bank-aligned, and the copy fills the exp-wait
                        # bubble on the engines.  drain_O(rp) only needs
                        # pair rp, which is always >= 1 round behind.
                        vp = psS.tile([MB, 2, HALF], f32, tag="S")
                        for pl in range(2):
                            t = 2 * r + pl
                            ms = slice(t * MB, (t + 1) * MB)
                            rows = (slice(0, CIN) if t % 2 == 0
                                    else slice(CIN, 2 * CIN))
                            nc.tensor.matmul(vp[:, pl, 0:C],
                                             x2_sb[rows, ms], wv2_sb[rows, :],
                                             start=True, stop=True)
                        dst = vT_sb[:, r, :, 0:C]
                        if r % 2 == 1:
                            nc.vector.tensor_copy(dst, vp[:, :, 0:C])
                        else:
                            nc.scalar.activation(out=dst, in_=vp[:, :, 0:C],
                                                 func=AF.Copy)
                    if r >= 2 and pending_stages:
                        pending_stages.pop(0)()
                    if abl == "sonly":
                        continue
                    e8 = epool.tile([MB, 2, NB], fp8, tag="E")
                    emit_exp(s_psA, tA, e8)
                    emit_exp(s_psB, tB, e8)
                    if abl == "noacc":
                        continue
                    pend_O.append((r, e8))
                    # O emission lags S; block j>0 additionally waits until
                    # epilogue(j-1) is fully emitted (psO slot reuse order)
                    if r >= 1 and not pending_stages:
                        drain_O(1 if len(pend_O) <= 3 else 2)
                while pending_stages:
                    pending_stages.pop(0)()
                drain_O(len(pend_O))
                if abl in ("sonly", "noacc", "noepi", "noepic"):
                    continue
                if j == 0:
                    if abl == "nounary":
                        xpu = xf_sb
                    else:
                        xpu = emit_unary()
                pending_stages = make_epilogue(j, o_ps)
            while pending_stages:
                pending_stages.pop(0)()
            if abl in ("sonly", "noacc", "noepi", "noepic", "noout"):
                # dummy output so the program has a live result
                nc.sync.dma_start(out=out_d[:, 0:NB], in_=xf_sb[:, 0:NB])

    if compat:
        _fix_walrus_compat(nc)
    return nc


def _fix_walrus_compat(nc):
    """Work around version skew between concourse and this walrus build.

    1. This walrus accepts at most ONE sync wait per instruction
       (setupSyncWait: "Too many sync wait commands").  Excess waits move
       to same-engine NOPs inserted immediately before the instruction --
       engine program order preserves the wait-before-execute semantics.
    2. EVENT_SEMAPHORE_RANGE_CLEAR (emitted by TileContext exit to reset
       tile semaphores) has a different ISA struct length in this walrus
       ("ISA wrong length").  Replace with one NOP per semaphore carrying
       a sem-wr-imm 0 update.
    """
    from concourse import mybir

    for f in nc.m.functions:
        for blk in f.blocks:
            new = []
            for inst in blk.instructions:
                si = inst.sync_info
                if (type(inst).__name__ == "InstISA"
                        and getattr(inst, "op_name", None)
                        == "EVENT_SEMAPHORE_RANGE_CLEAR"):
                    d = inst.ant_dict
                    first, last = d["range_first"], d["range_last"]
                    waits = list(si.on_wait) if si else []
                    for s in range(first, last + 1):
                        upd = mybir.SyncUpdate(
                            sync_type="semaphore", id=s,
                            ant_name=f"semreset_{s}",
                            update_mode="sem-wr-imm", update_value=0,
                            update_reg=None)
                        nop = mybir.InstNoOp(
                            name=f"semreset_{nc.next_id()}",
                            sync_info=mybir.SyncInfo(
                                on_wait=[waits.pop()] if waits else [],
                                on_update=[upd]),
                            bass_nofuse=True,
                            engine=inst.engine)
                        new.append(nop)
                    while waits:
                        nop = mybir.InstNoOp(
                            name=f"semreset_{nc.next_id()}",
                            sync_info=mybir.SyncInfo(
                                on_wait=[waits.pop()], on_update=[]),
                            bass_nofuse=True, engine=inst.engine)
                        new.insert(0, nop)
                    continue
                if si is not None and len(si.on_wait) > 1:
                    waits = list(si.on_wait)
                    excess, keep = waits[:-1], waits[-1:]
                    for w in excess:
                        nop = mybir.InstNoOp(
                            name=f"mwfix_{nc.next_id()}",
                            sync_info=mybir.SyncInfo(on_wait=[w], on_update=[]),
                            bass_nofuse=True,
                            engine=inst.engine)
                        new.append(nop)
                    inst.sync_info = mybir.SyncInfo(
                        on_wait=keep, on_update=list(si.on_update))
                new.append(inst)
            blk.instructions[:] = new


def _prep_inputs(x, wq, bq, wk, bk, wv, bv):
    """Host-side shard prep: per-core input maps (batch i -> core i)."""
    import ml_dtypes
    bf = ml_dtypes.bfloat16
    x = np.asarray(x, np.float32)
    wqT = np.asarray(wq, np.float32).T          # [cin, c]
    wkT = np.asarray(wk, np.float32).T
    wvT = np.asarray(wv, np.float32).T
    wqk = np.zeros((2 * CIN, 2 * C), np.float32)
    wqk[0:CIN, 0:C] = wqT
    wqk[0:CIN, C:2 * C] = wqT
    wqk[CIN:2 * CIN, 0:C] = wkT
    wqk[CIN:2 * CIN, C:2 * C] = wkT
    wv2 = np.concatenate([wvT, wvT], 0)          # [128, 64]
    bq2 = np.concatenate([np.asarray(bq, np.float32)] * 2)[:, None]
    bvx = (2.0 * np.asarray(bv, np.float32))[:, None]
    wqk_bf = wqk.astype(bf)
    wv2_bf = wv2.astype(bf)
    maps = []
    for i in range(B):
        xi = x[i].reshape(CIN, N)
        x2 = np.concatenate([xi, xi], 0).astype(bf)
        maps.append({
            "x2": np.ascontiguousarray(x2),
            "xf": np.ascontiguousarray(xi),
            "wqk": wqk_bf, "wv2": wv2_bf,
            "bq2": np.ascontiguousarray(bq2),
            "bvx": np.ascontiguousarray(bvx),
        })
    return maps


def kernel(x, wq, bq, wk, bk, wv, bv):
    from concourse.bass_utils import run_bass_kernel_spmd

    if "nc" not in _CACHE:
        _CACHE["nc"] = _build()
    nc = _CACHE["nc"]
    in_maps = _prep_inputs(x, wq, bq, wk, bk, wv, bv)
    res = run_bass_kernel_spmd(nc, in_maps, list(range(B)))
    out = np.stack([res.results[i]["out"].reshape(C, H, W) for i in range(B)])
    return out.astype(np.float32)



# revision 19
# speedup vs baseline: 1.5272x; 1.5272x over previous
"""Trainium2 Bass kernel for the disentangled non-local block.

Per batch b (one NeuronCore each, 8 batches over 8 cores):
  x: [64, 4096]; q/k/v = 1x1 conv GEMMs; out = x + softmax(q'k/8)V + unary.

Design (measured ~166us/iter on HW vs ~267us for the fp32r baseline):
  - All-bf16 GEMM datapath.  Biases are algebraically removed: bq/bk
    cancel in the softmax over keys (only qbar keeps bq); bv contributes
    exactly +2*bv[c] per output pixel (b_v*(1^T E) = b_v*D cancels /D).
  - x, q, k duplicated onto partitions 64..127 so S matmuls run as
    row-packed concurrent pairs (two 64-contraction tiles at once via
    tile_position auto-derive) -- 2x PE throughput on the S stream.
    The duplication is free: GEMM weights are stored twice ([wq|wq]).
  - exp of the [128 m, 1024 n] logit tiles split across ScalarE (native
    Exp with per-partition bias -u_m/8) and DVE via a one-instruction
    Schraudolph fast-exp: int8 out = s*1.44269 + (56.0368 - 1.44269*u_m)
    whose bits are the fp8e4m3 approximation of exp((s-u_m)/8);
    zero-mean error that cancels through the softmax normalization.
    (GPSIMD cannot read PSUM on this HW, so Pool takes no exp tiles.)
  - O accumulation via fp8e4m3 DoubleRow matmuls: vT stored interleaved
    [128, pair, 2, 80] with chunk pairs 2-per-cell, so each O matmul
    contracts 256 keys in 512 cycles -- 2x the bf16 O throughput.  The
    ones column of vT (at index 64) accumulates the softmax denominator
    at PSUM partition 64 for free.  The unary path reuses the fp8 vT
    with eu scaled by 1/16 (a common scale cancels in out_u/du).
  - Single PSUM O accumulator per n-block (psS 3x[128,1024] + psO 1):
    O-matmul emission lags S/exp and waits for the staged epilogue of
    the previous block, so the PE queue never head-of-line blocks.
  - Epilogue staged across the next block's rounds (recip+bcast mm,
    bcs copy, mul+add+DMA) to keep the DVE queue from stalling; the
    last block runs a 512-wide minimum-latency tail instead.
  - Big SBUF tiles double-buffered across the repeat loop so back-to-
    back iterations overlap (repeat-delta HW timing measures this).
"""

import numpy as np

B = 8
CIN = 64
C = 64
H = W = 64
N = H * W            # 4096
NB = 1024            # n-block (columns per outer iteration)
NBLK = N // NB       # 4
MB = 128             # m-chunk (keys per matmul, partition dim)
MCH = N // MB        # 32
HALF = 512           # PSUM bank free-dim limit for fp32 matmul output
SCALE = 0.125        # 1 / (sqrt(C) * temperature)
C0S = 23.0831        # SCALE * 2^7 / ln2   (Schraudolph slope, bf16)
C1S = 16249.08       # 127*2^7 + 0.5 - 0.0579*2^7 (trunc + centering)
C0S8 = 1.44269       # SCALE * 2^3 / ln2   (Schraudolph slope, fp8e4m3)
C1S8 = 56.0368       # 7*2^3 + 0.5 - 0.0579*2^3

# exp-tile engine assignment weights (ACT, DVE, POOL) out of the 128
# [128,1024] logit tiles.  Tuned so ACT and DVE busy-times balance.
import os as _os
W_ACT = int(_os.environ.get("W_ACT", "74"))
W_DVE = int(_os.environ.get("W_DVE", str(128 - int(_os.environ.get("W_ACT", "74")))))
W_POOL = 0   # Pool can't read PSUM on HW

_CACHE = {}
ABLATE = ""


def _exp_pattern():
    """Deterministic interleave of 'A'/'D'/'P' with the global ratios."""
    pat = []
    cnt = {"A": 0, "D": 0, "P": 0}
    wts = {"A": W_ACT, "D": W_DVE, "P": W_POOL}
    tot = W_ACT + W_DVE + W_POOL
    for i in range(128):
        best = max("ADP", key=lambda e: wts[e] * (i + 1) / tot - cnt[e])
        cnt[best] += 1
        pat.append(best)
    return pat


def _build(repeat=1, compat=True, ablate=None):
    import concourse.bass as bass
    import concourse.tile as tile
    from concourse import mybir

    f32 = mybir.dt.float32
    bf16 = mybir.dt.bfloat16
    i16 = mybir.dt.int16
    fp8 = mybir.dt.float8e4
    i8 = mybir.dt.int8
    AX = mybir.AxisListType
    AF = mybir.ActivationFunctionType
    ALU = mybir.AluOpType

    abl = ABLATE if ablate is None else ablate
    out_eng_name = _os.environ.get("OUT_Q", "sync")
    nc = bass.Bass()
    x2_d = nc.dram_tensor("x2", [2 * CIN, N], bf16, kind="ExternalInput")
    xf_d = nc.dram_tensor("xf", [C, N], f32, kind="ExternalInput")
    wqk_d = nc.dram_tensor("wqk", [2 * CIN, 2 * C], bf16, kind="ExternalInput")
    wv2_d = nc.dram_tensor("wv2", [2 * CIN, C], bf16, kind="ExternalInput")
    bq2_d = nc.dram_tensor("bq2", [2 * C, 1], f32, kind="ExternalInput")
    bvx_d = nc.dram_tensor("bvx", [C, 1], f32, kind="ExternalInput")
    out_d = nc.dram_tensor("out", [C, N], f32, kind="ExternalOutput")

    pat = _exp_pattern()

    with tile.TileContext(nc) as tc:
        with (
            tc.tile_pool(name="sing", bufs=1) as sing,
            tc.tile_pool(name="dbl", bufs=2) as dbl,
            tc.tile_pool(name="epool", bufs=12) as epool,
            tc.tile_pool(name="dpool", bufs=2) as dpool,
            tc.tile_pool(name="opool", bufs=2) as opool,
            tc.tile_pool(name="psS", bufs=3, space="PSUM") as psS,
            tc.tile_pool(name="psO", bufs=1, space="PSUM") as psO,
        ):
          for _rep in range(repeat):
            # ---- input DMAs ----
            # All inputs ride the gpsimd queue, which is idle from early
            # in each iteration: the NEXT iteration's input DMAs therefore
            # prefetch into the double-buffered tiles while this iteration
            # computes, and never queue behind the output DMAs (sync).
            # Emission order = need order (wqk gates the first GEMM).
            wqk_sb = sing.tile([2 * CIN, 2 * C], bf16)
            wv2_sb = sing.tile([2 * CIN, C], bf16)
            bq2_sb = sing.tile([2 * C, 1], f32)
            bvx_sb = sing.tile([C, 1], f32)
            x2_sb = dbl.tile([2 * CIN, N], bf16, tag="x2_sb")
            xf_sb = dbl.tile([C, N], f32, tag="xf_sb")
            nc.gpsimd.dma_start(out=wqk_sb, in_=wqk_d[:])
            nc.gpsimd.dma_start(out=x2_sb[:, 0:N // 4], in_=x2_d[:, 0:N // 4])
            nc.gpsimd.dma_start(out=bq2_sb, in_=bq2_d[:])
            for qd in range(1, 4):
                qs = slice(qd * (N // 4), (qd + 1) * (N // 4))
                nc.gpsimd.dma_start(out=x2_sb[:, qs], in_=x2_d[:, qs])
            nc.gpsimd.dma_start(out=wv2_sb, in_=wv2_d[:])
            nc.gpsimd.dma_start(out=bvx_sb, in_=bvx_d[:])
            # xf loaded FOLDED: n-half 0 on partitions 0:64, n-half 1 on
            # partitions 64:128 -- the +x adds and the xpu broadcast then
            # run on all 128 lanes (free-dim-bound DVE/ACT ops halve).
            for h in range(2):
                nc.gpsimd.dma_start(
                    out=xf_sb[h * C:(h + 1) * C, :],
                    in_=xf_d[:, h * (N // 2):(h + 1) * (N // 2)])
            # ---- input-independent init, emitted first so DVE/ACT work
            # while the input DMAs are in flight.  The dummy activation
            # pre-loads the ACT function table (Exp/Copy/Identity share
            # one set) so the first real copy doesn't pay the ~1.3us
            # table load.
            ones_sb = sing.tile([1, C], bf16)
            nc.vector.memset(ones_sb, 1.0)
            vT_sb = dbl.tile([MB, MCH // 2, 2, 80], fp8, tag="vT_sb")
            if _os.environ.get("NARROW_MEMSET", "1") == "1":
                nc.vector.memset(vT_sb[:, :, :, C:C + 1], 1.0)
            else:
                nc.vector.memset(vT_sb, 1.0)
            eub = sing.tile([MB, 1], f32)
            nc.vector.memset(eub, -2.772589)
            qmean2 = sing.tile([MB, 2], bf16)
            nc.vector.memset(qmean2, 0.0)
            if _os.environ.get("WARM", "1") == "1":
                warm_sb = sing.tile([1, 1], f32)
                nc.scalar.activation(out=warm_sb, in_=ones_sb[0:1, 0:1],
                                     func=AF.Exp)

            # ---- q,k GEMMs: row-packed concurrent pairs ----
            # rows 0-63 of wqk = [wqT|wqT]  -> q duplicated on 128 parts
            # rows 64-127     = [wkT|wkT]   -> k duplicated
            # q first (all of qbar's deps land early -> ubias -> first exp
            # starts sooner); copies merged to 1024-wide (2-bank PSUM
            # tiles) to halve per-op overheads.
            q2_sb = dbl.tile([MB, N], bf16, tag="q2_sb")
            k2_sb = dbl.tile([MB, N], bf16, tag="k2_sb")
            qsum8 = dbl.tile([MB, 4], f32, tag="qsum8")
            for h in range(N // NB):
                hs = slice(h * NB, (h + 1) * NB)
                qp = psS.tile([MB, NB], f32, tag="S")
                for g in range(2):
                    gs = slice(h * NB + g * HALF, h * NB + (g + 1) * HALF)
                    nc.tensor.matmul(qp[:, g * HALF:(g + 1) * HALF],
                                     wqk_sb[0:CIN, :], x2_sb[0:CIN, gs],
                                     start=True, stop=True)
                # PSUM->SBUF bf16 copies, alternated across ACT/DVE so
                # neither engine serializes the copy phase; both paths
                # accumulate the row-sum for qbar.
                if h % 2 == 0:
                    nc.scalar.activation(out=q2_sb[:, hs], in_=qp,
                                         func=AF.Copy,
                                         accum_out=qsum8[:, h:h + 1])
                else:
                    nc.vector.tensor_scalar(
                        out=q2_sb[:, hs], in0=qp, scalar1=1.0, scalar2=0.0,
                        op0=ALU.mult, op1=ALU.add,
                        accum_out=qsum8[:, h:h + 1])
            # qbar right after the q phase so the u-matmul inputs are
            # ready by the time the k phase drains.
            qsum = sing.tile([MB, 1], f32)
            nc.vector.reduce_sum(qsum, qsum8, axis=AX.X)
            nc.vector.tensor_scalar(
                out=qmean2[:, 0:1], in0=qsum, scalar1=1.0 / N,
                scalar2=bq2_sb, op0=ALU.mult, op1=ALU.add)

            # ---- k GEMMs with the v GEMMs interleaved ----
            # v chunks ride the gaps of the k copy cadence instead of
            # forming a serial ~6us PE/DVE phase before the S loop.
            for h in range(N // NB):
                hs = slice(h * NB, (h + 1) * NB)
                kp = psS.tile([MB, NB], f32, tag="S")
                for g in range(2):
                    gs = slice(h * NB + g * HALF, h * NB + (g + 1) * HALF)
                    nc.tensor.matmul(kp[:, g * HALF:(g + 1) * HALF],
                                     wqk_sb[CIN:2 * CIN, :],
                                     x2_sb[CIN:2 * CIN, gs],
                                     start=True, stop=True)
                if h % 2 == 0:
                    nc.vector.tensor_copy(k2_sb[:, hs], kp)
                else:
                    nc.scalar.activation(out=k2_sb[:, hs], in_=kp,
                                         func=AF.Copy)
                pass  # v GEMMs are emitted inside block 0's rounds

            # ---- unary logits u = qbar . k ----
            u_ps = psS.tile([MB, 2 * MCH], f32, tag="S")
            for t in range(MCH):
                ms = slice(t * MB, (t + 1) * MB)
                rows = slice(0, CIN) if t % 2 == 0 else slice(CIN, 2 * CIN)
                nc.tensor.matmul(
                    u_ps[:, 2 * t:2 * t + 2], k2_sb[rows, ms],
                    qmean2[rows, :], start=True, stop=True)
            u_even = u_ps[:].rearrange("p (t two) -> p t two", two=2)[:, :, 0:1]
            # exp biases: ACT wants -SCALE*u ; DVE/Pool want C1S - C0S*u
            ubias_a = dbl.tile([MB, MCH], f32, tag="ubias_a")
            nc.scalar.mul(ubias_a, u_even, -SCALE)
            ubias_d = dbl.tile([MB, MCH], f32, tag="ubias_d")
            nc.vector.tensor_scalar(
                out=ubias_d, in0=u_even, scalar1=-C0S8, scalar2=C1S8,
                op0=ALU.mult, op1=ALU.add)
            eu = dbl.tile([MB, 2 * MCH], fp8, tag="eu")
            nc.scalar.activation(out=eu, in_=u_ps, func=AF.Exp,
                                 bias=eub)  # exp(u)/16: du scale cancels


            # ---- unary epilogue (emitted right after block 0) ----
            def emit_unary():
                uacc = psS.tile([C + 1, 2], f32, tag="S")
                for t in range(MCH):
                    nc.tensor.matmul(
                        uacc, vT_sb[:, t // 2, t % 2, 0:C + 1],
                        eu[:, 2 * t:2 * t + 2],
                        start=(t == 0), stop=(t == MCH - 1))
                du = sing.tile([1, 1], f32)
                nc.vector.tensor_copy(du, uacc[C:C + 1, 0:1])
                recu = sing.tile([1, 1], f32)
                nc.vector.reciprocal(recu, du)
                bcu = sing.tile([C, 1], f32)
                rau = recu[:]
                nc.sync.dma_start(out=bcu, in_=bass.AP(
                    tensor=rau.tensor, offset=rau.offset,
                    ap=[rau.ap[0], [0, C], [1, 1]]))
                ucp = sing.tile([C, 1], f32)
                nc.vector.tensor_copy(ucp, uacc[0:C, 0:1])
                ub_sb = sing.tile([C, 1], f32)
                nc.vector.tensor_mul(ub_sb, ucp, bcu)
                ubv = sing.tile([C, 1], f32)
                nc.vector.tensor_add(ubv, ub_sb, bvx_sb)
                # xpu = x + (unary + 2bv), broadcast along n (ACT:
                # real-HW gpsimd elementwise is far slower than modeled)
                xpu = sing.tile([C, N], f32)
                nc.scalar.activation(out=xpu, in_=xf_sb, func=AF.Identity,
                                     bias=ubv, scale=1.0)
                return xpu

            # ---- main loop ----
            out_sb = dbl.tile([C, N], f32, tag="out")
            xpu = None
            expi = [0]       # global exp-tile counter (for pattern)

            def emit_exp(s_ps, t, e8):
                pl = t % 2
                eng = pat[expi[0] % 128]
                expi[0] += 1
                if eng == "A":
                    nc.scalar.activation(out=e8[:, pl, :], in_=s_ps,
                                         func=AF.Exp, scale=SCALE,
                                         bias=ubias_a[:, t:t + 1])
                else:
                    nc.vector.tensor_scalar(
                        out=e8.bitcast(i8)[:, pl, :], in0=s_ps, scalar1=C0S8,
                        scalar2=ubias_d[:, t:t + 1],
                        op0=ALU.mult, op1=ALU.add)

            def make_epilogue(j, o_ps):
                """Stage list; each stage is one engine-chain step so the
                DVE queue never head-of-line-blocks on a long chain."""
                def st_recip():
                    rec = dpool.tile([1, NB], bf16, tag="rec")
                    with nc.allow_low_precision(reason="1/D bcast bf16"):
                        nc.vector.reciprocal(rec, o_ps[C:C + 1, :])
                    for h in range(NB // HALF):
                        hs = slice(h * HALF, (h + 1) * HALF)
                        nc.tensor.matmul(
                            o_ps[C:2 * C, hs], ones_sb, rec[:, hs],
                            start=True, stop=True, tile_position=(0, C))

                def st_bcs():
                    bcs = opool.tile([C, NB], f32, tag="bcs")
                    nc.vector.tensor_copy(bcs, o_ps[C:2 * C, :])
                    st_bcs.bcs = bcs

                def st_fin():
                    gs = slice(j * NB, (j + 1) * NB)
                    o_sb = out_sb[:, gs]
                    nc.vector.tensor_mul(o_sb, o_ps[0:C, :], st_bcs.bcs)
                    nc.vector.tensor_add(o_sb, o_sb, xpu[:, gs])
                    if abl != "noout":
                        getattr(nc, out_eng_name).dma_start(out=out_d[:, gs], in_=o_sb)

                def tail():
                    # last block: two 512-half chains interleaved; the bcs
                    # copies go to ACT (idle at this point) so the DVE
                    # serial chain is recip/mul/add only.
                    hs_l = [slice(h * HALF, (h + 1) * HALF)
                            for h in range(NB // HALF)]
                    gs_l = [slice(j * NB + h * HALF, j * NB + (h + 1) * HALF)
                            for h in range(NB // HALF)]
                    recs, bcss = [], []
                    for h in range(NB // HALF):
                        rec = dpool.tile([1, HALF], bf16, tag="rec")
                        with nc.allow_low_precision(reason="1/D bcast bf16"):
                            nc.vector.reciprocal(rec, o_ps[C:C + 1, hs_l[h]])
                        recs.append(rec)
                        nc.tensor.matmul(
                            o_ps[C:2 * C, hs_l[h]], ones_sb, rec,
                            start=True, stop=True, tile_position=(0, C))
                    for h in range(NB // HALF):
                        bcs = opool.tile([C, HALF], f32, tag="bcs")
                        nc.scalar.activation(out=bcs, in_=o_ps[C:2 * C, hs_l[h]],
                                             func=AF.Copy)
                        bcss.append(bcs)
                    for h in range(NB // HALF):
                        o_sb = out_sb[:, gs_l[h]]
                        nc.vector.tensor_mul(o_sb, o_ps[0:C, hs_l[h]], bcss[h])
                        nc.vector.tensor_add(o_sb, o_sb, xpu[:, gs_l[h]])
                        if abl != "noout":
                            getattr(nc, out_eng_name).dma_start(out=out_d[:, gs_l[h]], in_=o_sb)

                if j == NBLK - 1:
                    return [tail]
                return [st_recip, st_bcs, st_fin]

            pending_stages = []
            for j in range(NBLK):
                o_ps = None
                pend_O = []      # (t, e_sb) waiting for O emission
                first_t = [True]

                def drain_O(n=2):
                    nonlocal o_ps
                    for _ in range(min(n, len(pend_O))):
                        rp, e8 = pend_O.pop(0)
                        if o_ps is None:
                            o_ps = psO.tile([MB, NB], f32, tag="O")
                        for h in range(NB // HALF):
                            hs = slice(h * HALF, (h + 1) * HALF)
                            nc.tensor.matmul(
                                o_ps[0:C + 1, hs],
                                vT_sb[:, rp, :, 0:C + 1], e8[:, :, hs],
                                start=first_t[0], stop=(rp == MCH // 2 - 1),
                                perf_mode=mybir.MatmulPerfMode.DoubleRow)
                        first_t[0] = False

                for r in range(MCH // 2):
                    tA, tB = 2 * r, 2 * r + 1
                    s_psA = psS.tile([MB, NB], f32, tag="S")
                    s_psB = psS.tile([MB, NB], f32, tag="S")
                    for h in range(NB // HALF):
                        hs = slice(h * HALF, (h + 1) * HALF)
                        nc.tensor.matmul(
                            s_psA[:, hs], k2_sb[0:CIN, tA * MB:(tA + 1) * MB],
                            q2_sb[0:CIN, j * NB + h * HALF:
                                  j * NB + (h + 1) * HALF],
                            start=True, stop=True)
                        nc.tensor.matmul(
                            s_psB[:, hs],
                            k2_sb[CIN:2 * CIN, tB * MB:(tB + 1) * MB],
                            q2_sb[CIN:2 * CIN, j * NB + h * HALF:
                                  j * NB + (h + 1) * HALF],
                            start=True, stop=True)
                    if j == 0:
                        # v chunk pair r rides round r: its PSUM tile takes
                        # the same ring slot every round, its matmuls are
                        # bank-aligned, and the copy fills the exp-wait
                        # bubble on the engines.  drain_O(rp) only needs
                        # pair rp, which is always >= 1 round behind.
                        vp = psS.tile([MB, 2, HALF], f32, tag="S")
                        for pl in range(2):
                            t = 2 * r + pl
                            ms = slice(t * MB, (t + 1) * MB)
                            rows = (slice(0, CIN) if t % 2 == 0
                                    else slice(CIN, 2 * CIN))
                            nc.tensor.matmul(vp[:, pl, 0:C],
                                             x2_sb[rows, ms], wv2_sb[rows, :],
                                             start=True, stop=True)
                        dst = vT_sb[:, r, :, 0:C]
                        if r % 2 == 1:
                            nc.vector.tensor_copy(dst, vp[:, :, 0:C])
                        else:
                            nc.scalar.activation(out=dst, in_=vp[:, :, 0:C],
                                                 func=AF.Copy)
                    if r >= 2 and pending_stages:
                        pending_stages.pop(0)()
                    if abl == "sonly":
                        continue
                    e8 = epool.tile([MB, 2, NB], fp8, tag="E")
                    emit_exp(s_psA, tA, e8)
                    emit_exp(s_psB, tB, e8)
                    if abl == "noacc":
                        continue
                    pend_O.append((r, e8))
                    # O emission lags S; block j>0 additionally waits until
                    # epilogue(j-1) is fully emitted (psO slot reuse order)
                    if r >= 1 and not pending_stages:
                        drain_O(1 if len(pend_O) <= 3 else 2)
                while pending_stages:
                    pending_stages.pop(0)()
                drain_O(len(pend_O))
                if abl in ("sonly", "noacc", "noepi", "noepic"):
                    continue
                if j == 0:
                    if abl == "nounary":
                        xpu = xf_sb
                    else:
                        xpu = emit_unary()
                pending_stages = make_epilogue(j, o_ps)
            while pending_stages:
                pending_stages.pop(0)()
            if abl in ("sonly", "noacc", "noepi", "noepic", "noout"):
                # dummy output so the program has a live result
                nc.sync.dma_start(out=out_d[:, 0:NB], in_=xf_sb[:, 0:NB])

    if compat:
        _fix_walrus_compat(nc)
    return nc


def _fix_walrus_compat(nc):
    """Work around version skew between concourse and this walrus build.

    1. This walrus accepts at most ONE sync wait per instruction
       (setupSyncWait: "Too many sync wait commands").  Excess waits move
       to same-engine NOPs inserted immediately before the instruction --
       engine program order preserves the wait-before-execute semantics.
    2. EVENT_SEMAPHORE_RANGE_CLEAR (emitted by TileContext exit to reset
       tile semaphores) has a different ISA struct length in this walrus
       ("ISA wrong length").  Replace with one NOP per semaphore carrying
       a sem-wr-imm 0 update.
    """
    from concourse import mybir

    for f in nc.m.functions:
        for blk in f.blocks:
            new = []
            for inst in blk.instructions:
                si = inst.sync_info
                if (type(inst).__name__ == "InstISA"
                        and getattr(inst, "op_name", None)
                        == "EVENT_SEMAPHORE_RANGE_CLEAR"):
                    d = inst.ant_dict
                    first, last = d["range_first"], d["range_last"]
                    waits = list(si.on_wait) if si else []
                    for s in range(first, last + 1):
                        upd = mybir.SyncUpdate(
                            sync_type="semaphore", id=s,
                            ant_name=f"semreset_{s}",
                            update_mode="sem-wr-imm", update_value=0,
                            update_reg=None)
                        nop = mybir.InstNoOp(
                            name=f"semreset_{nc.next_id()}",
                            sync_info=mybir.SyncInfo(
                                on_wait=[waits.pop()] if waits else [],
                                on_update=[upd]),
                            bass_nofuse=True,
                            engine=inst.engine)
                        new.append(nop)
                    while waits:
                        nop = mybir.InstNoOp(
                            name=f"semreset_{nc.next_id()}",
                            sync_info=mybir.SyncInfo(
                                on_wait=[waits.pop()], on_update=[]),
                            bass_nofuse=True, engine=inst.engine)
                        new.insert(0, nop)
                    continue
                if si is not None and len(si.on_wait) > 1:
                    waits = list(si.on_wait)
                    excess, keep = waits[:-1], waits[-1:]
                    for w in excess:
                        nop = mybir.InstNoOp(
                            name=f"mwfix_{nc.next_id()}",
                            sync_info=mybir.SyncInfo(on_wait=[w], on_update=[]),
                            bass_nofuse=True,
                            engine=inst.engine)
                        new.append(nop)
                    inst.sync_info = mybir.SyncInfo(
                        on_wait=keep, on_update=list(si.on_update))
                new.append(inst)
            blk.instructions[:] = new


def _prep_inputs(x, wq, bq, wk, bk, wv, bv):
    """Host-side shard prep: per-core input maps (batch i -> core i)."""
    import ml_dtypes
    bf = ml_dtypes.bfloat16
    x = np.asarray(x, np.float32)
    wqT = np.asarray(wq, np.float32).T          # [cin, c]
    wkT = np.asarray(wk, np.float32).T
    wvT = np.asarray(wv, np.float32).T
    wqk = np.zeros((2 * CIN, 2 * C), np.float32)
    wqk[0:CIN, 0:C] = wqT
    wqk[0:CIN, C:2 * C] = wqT
    wqk[CIN:2 * CIN, 0:C] = wkT
    wqk[CIN:2 * CIN, C:2 * C] = wkT
    wv2 = np.concatenate([wvT, wvT], 0)          # [128, 64]
    bq2 = np.concatenate([np.asarray(bq, np.float32)] * 2)[:, None]
    bvx = (2.0 * np.asarray(bv, np.float32))[:, None]
    wqk_bf = wqk.astype(bf)
    wv2_bf = wv2.astype(bf)
    maps = []
    for i in range(B):
        xi = x[i].reshape(CIN, N)
        x2 = np.concatenate([xi, xi], 0).astype(bf)
        maps.append({
            "x2": np.ascontiguousarray(x2),
            "xf": np.ascontiguousarray(xi),
            "wqk": wqk_bf, "wv2": wv2_bf,
            "bq2": np.ascontiguousarray(bq2),
            "bvx": np.ascontiguousarray(bvx),
        })
    return maps


def kernel(x, wq, bq, wk, bk, wv, bv):
    from concourse.bass_utils import run_bass_kernel_spmd

    if "nc" not in _CACHE:
        _CACHE["nc"] = _build()
    nc = _CACHE["nc"]
    in_maps = _prep_inputs(x, wq, bq, wk, bk, wv, bv)
    res = run_bass_kernel_spmd(nc, in_maps, list(range(B)))
    out = np.stack([res.results[i]["out"].reshape(C, H, W) for i in range(B)])
    return out.astype(np.float32)

